# revision 12
# baseline (speedup 1.0000x reference)
"""Trainium2 Bass kernel for nn_BatchMoEProcessor.

Strategy (see spec sharding_hint): data-parallel over the cell batch dim
across 8 NeuronCores (2048 cells/core); expert + gating weights replicated.
full_lattice_states is sharded by relevance: for every group of 256 cells we
build the shard of lattice rows that group's neighbor/cell indices reference
(<= 19201 unique rows, int16-addressable) and remap the index tensors into
that shard; the device resolves all neighbor reads with dma_gather (bulk
SWDGE gather, 4 queues) out of the shard tables in HBM.

Per core the device computes, entirely on-chip after the gathers:
  masked neighbor means (DVE tree reduction + per-cell 1/count scaling),
  three expert MLPs (PE matmuls, fp32, weight-stationary sweeps, tanh on ACT),
  softmax gating (PE + ACT/DVE), masked gate combine + residual
  (fused scalar_tensor_tensor), and writes its [2048, 256] output slice.
"""

import os
import sys

for _p in ("/opt/trn_rl_repo",):
    if _p not in sys.path:
        sys.path.insert(0, _p)

import numpy as np

import concourse.bass as bass
import concourse.bacc as bacc
import concourse.mybir as mybir
import concourse.tile as tile
from concourse import bass_utils
from concourse import library_config as library_config
from concourse.masks import make_identity

F32 = mybir.dt.float32
I16 = mybir.dt.int16
I32 = mybir.dt.int32
OP = mybir.AluOpType

N_CELLS = 100000
BATCH = 16384
D = 256
H = 512
KS = {"local": 26, "functional": 32, "distant": 16}
EXPERTS = ("local", "functional", "distant")

N_CORES = 8
BC = BATCH // N_CORES          # cells per core
GROUP = 256                     # cells per shard-table group
N_GROUPS = BC // GROUP          # 8 groups per core
TILES_PER_GROUP = GROUP // 128  # 2
N_TILES = BC // 128             # 16

UMAX = 19584                    # shard table rows (>= 256*75+1 worst case)
ZROW = UMAX - 1                 # all-zeros row, shared sentinel for masked slots

# gather call chunking: slots per dma_gather call (per 128-cell tile)
CHUNKS = {
    "local": (8, 8, 8, 2),
    "functional": (8, 8, 8, 8),
    "distant": (8, 8),
}


def _wrap_idx(v):
    """int16 vector [NI] -> dma_gather SBUF layout [128, NI/16]."""
    ni = v.shape[0]
    w = v.reshape(ni // 16, 16).T
    return np.tile(w, (8, 1))


def _build_core_inputs(core, cell_indices, full, idx, counts, weights):
    """Host-side sharding for one core: shard tables + remapped int16 indices."""
    sl = slice(core * BC, (core + 1) * BC)
    ci = cell_indices[sl]
    idx_c = {e: idx[e][sl] for e in EXPERTS}
    cnt_c = {e: counts[e][sl] for e in EXPERTS}

    tables = np.zeros((N_GROUPS, UMAX, D), dtype=np.float32)
    idx_cat = []
    for g in range(N_GROUPS):
        cs = slice(g * GROUP, (g + 1) * GROUP)
        refs = [ci[cs]]
        masked = {}
        for e in EXPERTS:
            K = KS[e]
            m = idx_c[e][cs].copy()
            invalid = np.arange(K)[None, :] >= cnt_c[e][cs][:, None]
            m[invalid] = -1
            masked[e] = m
            refs.append(m[m >= 0])
        uniq = np.unique(np.concatenate([r.ravel() for r in refs]))
        assert uniq.size <= ZROW, f"group unique overflow: {uniq.size}"
        tables[g, : uniq.size] = full[uniq]
        # remap helper: global row -> local table row; -1 -> ZROW
        def remap(a):
            out = np.searchsorted(uniq, np.maximum(a, 0)).astype(np.int16)
            out[a < 0] = ZROW
            return out

        cols = []
        for e in EXPERTS:
            loc = remap(masked[e])  # [GROUP, K]
            for t in range(TILES_PER_GROUP):
                rows = loc[t * 128 : (t + 1) * 128]  # [128, K]
                c0 = 0
                for csz in CHUNKS[e]:
                    blk = rows[:, c0 : c0 + csz]      # [128, csz]
                    c0 += csz
                    v = blk.T.reshape(-1)             # i = s*128 + p
                    cols.append(_wrap_idx(v))
        loc_cur = remap(ci[cs].astype(np.int64))
        for t in range(TILES_PER_GROUP):
            cols.append(_wrap_idx(loc_cur[t * 128 : (t + 1) * 128]))
        idx_cat.append(np.concatenate(cols, axis=1))
    idx_arr = np.stack(idx_cat)  # [N_GROUPS, 128, IDXW]

    inp = {"tables": tables, "idxs": idx_arr}
    for e in EXPERTS:
        inp[f"counts_{e}"] = cnt_c[e].reshape(N_TILES, 128).T.astype(np.int32).copy()
    inp.update(weights)
    return inp


def _prep_weights(inputs):
    w = {}
    for e in EXPERTS:
        w[f"W1_{e}"] = np.ascontiguousarray(inputs[f"W1_{e}"].reshape(4, 128, H))
        w[f"W2_{e}"] = np.ascontiguousarray(inputs[f"W2_{e}"].reshape(4, 128, D))
        w[f"b1_{e}"] = np.ascontiguousarray(inputs[f"b1_{e}"].reshape(4, 128).T)
        w[f"b2_{e}"] = np.ascontiguousarray(inputs[f"b2_{e}"].reshape(1, D))
    w["Wg"] = np.ascontiguousarray(inputs["Wg"].reshape(2, 128, 3))
    w["bg"] = np.ascontiguousarray(inputs["bg"].reshape(1, 3))
    return w


def build_model(n_tiles=N_TILES):
    """Build the per-core Bass program (same NEFF for all 8 cores)."""
    n_groups = n_tiles // TILES_PER_GROUP
    nc = bacc.Bacc(
        "TRN2",
        target_bir_lowering=False,
        debug=False,
        enable_asserts=False,
        num_devices=1,
        num_swdge_queues=4,
    )
    bc = n_tiles * 128

    tabs = nc.dram_tensor("tables", [n_groups, UMAX, D], F32, kind="ExternalInput").ap()
    idxw = sum(
        (128 * sum(CHUNKS[e]) // 16) * TILES_PER_GROUP for e in EXPERTS
    ) + 8 * TILES_PER_GROUP
    idxs = nc.dram_tensor("idxs", [n_groups, 128, idxw], I16, kind="ExternalInput").ap()
    cnts = {
        e: nc.dram_tensor(f"counts_{e}", [128, n_tiles], I32, kind="ExternalInput").ap()
        for e in EXPERTS
    }
    W1 = {e: nc.dram_tensor(f"W1_{e}", [4, 128, H], F32, kind="ExternalInput").ap() for e in EXPERTS}
    W2 = {e: nc.dram_tensor(f"W2_{e}", [4, 128, D], F32, kind="ExternalInput").ap() for e in EXPERTS}
    b1 = {e: nc.dram_tensor(f"b1_{e}", [128, 4], F32, kind="ExternalInput").ap() for e in EXPERTS}
    b2 = {e: nc.dram_tensor(f"b2_{e}", [1, D], F32, kind="ExternalInput").ap() for e in EXPERTS}
    Wg = nc.dram_tensor("Wg", [2, 128, 3], F32, kind="ExternalInput").ap()
    bg = nc.dram_tensor("bg", [1, 3], F32, kind="ExternalInput").ap()
    out = nc.dram_tensor("out", [bc, D], F32, kind="ExternalOutput").ap()

    nc.gpsimd.load_library(library_config.mlp)

    qn = [0]

    def next_q():
        q = qn[0] % 4
        qn[0] += 1
        return q

    with tile.TileContext(nc) as tc:
        with (
            tc.tile_pool(name="wpool", bufs=1) as wp,
            tc.tile_pool(name="wrot", bufs=3) as wrot,
            tc.tile_pool(name="small", bufs=1) as sp,
            tc.tile_pool(name="curp", bufs=1) as curp,
            tc.tile_pool(name="meanp", bufs=2) as meanp,
            tc.tile_pool(name="curtp", bufs=2) as curtp,
            tc.tile_pool(name="htp", bufs=1) as htp,
            tc.tile_pool(name="gbuf", bufs=3) as gbuf,
            tc.tile_pool(name="idxp", bufs=1) as idxp,
            tc.tile_pool(name="xtp", bufs=2) as xtp,
            tc.tile_pool(name="gatep", bufs=1) as gatep,
            tc.tile_pool(name="ps_tr", bufs=4, space="PSUM") as ps_tr,
            tc.tile_pool(name="ps_mm1", bufs=2, space="PSUM") as ps_mm1,
            tc.tile_pool(name="ps_mm2", bufs=2, space="PSUM") as ps_mm2,
        ):
            # ---- weights / constants into SBUF ----
            b1t = {}
            b2t = {}
            for e in EXPERTS:
                b1t[e] = wp.tile([128, 4], F32, tag=f"b1{e}", name=f"b1{e}")
                nc.sync.dma_start(out=b1t[e][:], in_=b1[e][:])
                b2t[e] = wp.tile([1, D], F32, tag=f"b2{e}", name=f"b2{e}")
                nc.sync.dma_start(out=b2t[e][:], in_=b2[e][:])
            wgt = [wp.tile([128, 3], F32, tag=f"wg{f}", name=f"wg{f}") for f in range(2)]
            for f in range(2):
                nc.sync.dma_start(out=wgt[f][:], in_=Wg[f])
            bgt = wp.tile([1, 3], F32, tag="bg")
            nc.sync.dma_start(out=bgt[:], in_=bg[:])
            ones_col = wp.tile([1, 128], F32, tag="ones")
            nc.vector.memset(ones_col[:], 1.0)
            ident = wp.tile([128, 128], F32, tag="ident")
            make_identity(nc, ident[:])

            # per-expert count scalars: inv = 1/max(c,1), mask = (c>0)
            invt = {}
            maskt = {}
            for e in EXPERTS:
                craw = sp.tile([128, n_tiles], I32, tag=f"craw{e}")
                nc.sync.dma_start(out=craw[:], in_=cnts[e][:, :n_tiles])
                cf = sp.tile([128, n_tiles], F32, tag=f"cf{e}")
                nc.vector.tensor_copy(out=cf[:], in_=craw[:])
                mx = sp.tile([128, n_tiles], F32, tag=f"mx{e}")
                nc.vector.tensor_scalar_max(out=mx[:], in0=cf[:], scalar1=1.0)
                iv = sp.tile([128, n_tiles], F32, tag=f"iv{e}")
                nc.vector.reciprocal(out=iv[:], in_=mx[:])
                mk = sp.tile([128, n_tiles], F32, tag=f"mk{e}")
                nc.vector.tensor_scalar(
                    out=mk[:], in0=cf[:], scalar1=0.0, scalar2=None, op0=OP.is_gt
                )
                invt[e] = iv
                maskt[e] = mk

            # idx column offsets within the per-group concat
            idx_off = {}
            off = 0
            for e in EXPERTS:
                for t in range(TILES_PER_GROUP):
                    c0 = 0
                    for j, csz in enumerate(CHUNKS[e]):
                        idx_off[(e, t, j)] = (off, csz * 8)
                        off += csz * 8
                        c0 += csz
            for t in range(TILES_PER_GROUP):
                idx_off[("cur", t, 0)] = (off, 8)
                off += 8
            assert off == idxw

            # ---- phase 0: load idx, gather cur, transpose cur, gates ----
            idx_tiles = []
            for g in range(n_groups):
                it = idxp.tile([128, idxw], I16, tag=f"idx{g}", name=f"idx{g}")
                nc.sync.dma_start(out=it[:], in_=idxs[g])
                idx_tiles.append(it)

            cur_tiles = []
            for t in range(n_tiles):
                g, tg = divmod(t, TILES_PER_GROUP)
                o, w_ = idx_off[("cur", tg, 0)]
                cur = curp.tile([128, D], F32, tag=f"cur{t}")
                nc.gpsimd.dma_gather(
                    out_ap=cur[:].rearrange("p (s d) -> p s d", d=D),
                    in_ap=tabs[g],
                    idxs_ap=idx_tiles[g][:, o : o + w_],
                    num_idxs=128,
                    num_idxs_reg=128,
                    elem_size=D,
                    queue_num=next_q(),
                )
                cur_tiles.append(cur)

            curT = [curtp.tile([128, 128 * n_tiles], F32, tag=f"curT{b}", name=f"curT{b}") for b in range(2)]
            for t in range(n_tiles):
                for b in range(2):
                    pt = ps_tr.tile([128, 128], F32, tag="tr", space="PSUM")
                    nc.tensor.transpose(
                        out=pt[:], in_=cur_tiles[t][:, b * 128 : (b + 1) * 128], identity=ident[:]
                    )
                    if (t + b) % 2 == 0:
                        nc.vector.tensor_copy(
                            out=curT[b][:, t * 128 : (t + 1) * 128], in_=pt[:]
                        )
                    else:
                        nc.scalar.activation(
                            out=curT[b][:, t * 128 : (t + 1) * 128], in_=pt[:],
                            func=mybir.ActivationFunctionType.Copy,
                        )

            # gates: softmax(cur @ Wg + bg)
            gate_tiles = []
            for t in range(n_tiles):
                pg = ps_mm2.tile([128, 3], F32, tag="mm2", space="PSUM")
                for b in range(2):
                    nc.tensor.matmul(
                        out=pg[:],
                        lhsT=curT[b][:, t * 128 : (t + 1) * 128],
                        rhs=wgt[b][:],
                        start=(b == 0),
                        stop=False,
                    )
                nc.tensor.matmul(
                    out=pg[:], lhsT=ones_col[:], rhs=bgt[:], start=False, stop=True
                )
                gmax = gatep.tile([128, 1], F32, tag="gm", bufs=2, name="gmax")
                nc.vector.reduce_max(out=gmax[:], in_=pg[:], axis=mybir.AxisListType.X)
                gsh = gatep.tile([128, 3], F32, tag="gs", bufs=2, name="gsh")
                nc.vector.scalar_tensor_tensor(
                    out=gsh[:], in0=pg[:], scalar=-1.0, in1=gmax[:].to_broadcast([128, 3]),
                    op0=OP.bypass, op1=OP.subtract,
                )
                gex = gatep.tile([128, 3], F32, tag="ge", bufs=2, name="gex")
                nc.scalar.activation(
                    out=gex[:], in_=gsh[:], func=mybir.ActivationFunctionType.Exp
                )
                gsum = gatep.tile([128, 1], F32, tag="gu", bufs=2, name="gsum")
                nc.vector.reduce_sum(out=gsum[:], in_=gex[:], axis=mybir.AxisListType.X)
                grec = gatep.tile([128, 1], F32, tag="gr", bufs=2, name="grec")
                nc.vector.reciprocal(out=grec[:], in_=gsum[:])
                gat = gatep.tile([128, 3], F32, tag=f"gg{t}", name=f"gg{t}")
                nc.vector.tensor_scalar(
                    out=gat[:], in0=gex[:], scalar1=grec[:, 0:1], scalar2=None, op0=OP.mult
                )
                gate_tiles.append(gat)

            # ---- per expert pipeline ----
            for e in EXPERTS:
                K = KS[e]
                w1cur = [wrot.tile([128, H], F32, tag=f"w1f{f}", name=f"w1f{f}") for f in range(4)]
                for f in range(4):
                    nc.sync.dma_start(out=w1cur[f][:], in_=W1[e][f])
                w2cur = [wrot.tile([128, D], F32, tag=f"w2m{m}", name=f"w2m{m}") for m in range(4)]
                for m in range(4):
                    nc.sync.dma_start(out=w2cur[m][:], in_=W2[e][m])
                # phase 1: gather + masked mean, per tile
                mean_tiles = []
                for t in range(n_tiles):
                    g, tg = divmod(t, TILES_PER_GROUP)
                    mean = meanp.tile([128, D], F32, tag=f"mean{t}", name=f"mean{t}")
                    for j, csz in enumerate(CHUNKS[e]):
                        o, w_ = idx_off[(e, tg, j)]
                        gb = gbuf.tile([128, 8 * D], F32, tag="g")
                        nc.gpsimd.dma_gather(
                            out_ap=gb[:, : csz * D].rearrange("p (s d) -> p s d", d=D),
                            in_ap=tabs[g],
                            idxs_ap=idx_tiles[g][:, o : o + w_],
                            num_idxs=csz * 128,
                            num_idxs_reg=csz * 128,
                            elem_size=D,
                            queue_num=next_q(),
                        )
                        # tree-reduce csz slots -> slot 0
                        size = csz
                        while size > 1:
                            half = size // 2
                            nc.vector.scalar_tensor_tensor(
                                out=gb[:, : half * D],
                                in0=gb[:, : half * D],
                                scalar=1.0,
                                in1=gb[:, half * D : 2 * half * D],
                                op0=OP.mult,
                                op1=OP.add,
                            )
                            if size % 2:
                                nc.vector.tensor_add(
                                    out=gb[:, :D],
                                    in0=gb[:, :D],
                                    in1=gb[:, (size - 1) * D : size * D],
                                )
                            size = half
                        if j == 0:
                            nc.vector.tensor_copy(out=mean[:], in_=gb[:, :D])
                        else:
                            nc.vector.tensor_add(
                                out=mean[:], in0=mean[:], in1=gb[:, :D]
                            )
                    nc.vector.tensor_scalar(
                        out=mean[:], in0=mean[:], scalar1=invt[e][:, t : t + 1], scalar2=None, op0=OP.mult
                    )
                    mean_tiles.append(mean)

                # phase 2: H.T = tanh(X.T' W1 + b1), X = [cur, mean]
                n_nb = n_tiles // 4 if n_tiles >= 4 else 1
                nb_w = n_tiles // n_nb  # tiles per cell-block
                ht = [htp.tile([128, 128 * n_tiles], F32, tag=f"ht{m}", name=f"htm{m}") for m in range(4)]
                for n in range(n_nb):
                    mt = []
                    for b in range(2):
                        x = xtp.tile([128, 128 * nb_w], F32, tag=f"xt{b}")
                        for tt in range(nb_w):
                            tglob = n * nb_w + tt
                            pt = ps_tr.tile(
                                [128, 128], F32, tag="tr", space="PSUM"
                            )
                            nc.tensor.transpose(
                                out=pt[:],
                                in_=mean_tiles[tglob][:, b * 128 : (b + 1) * 128],
                                identity=ident[:],
                            )
                            if (tglob + b) % 2 == 0:
                                nc.vector.tensor_copy(
                                    out=x[:, tt * 128 : (tt + 1) * 128], in_=pt[:]
                                )
                            else:
                                nc.scalar.activation(
                                    out=x[:, tt * 128 : (tt + 1) * 128], in_=pt[:],
                                    func=mybir.ActivationFunctionType.Copy,
                                )
                        mt.append(x)
                    cols = slice(n * nb_w * 128, (n + 1) * nb_w * 128)
                    for m in range(4):
                        ph = ps_mm1.tile(
                            [128, 128 * nb_w], F32, tag="mm1", space="PSUM"
                        )
                        for f in range(2):
                            nc.tensor.matmul(
                                out=ph[:],
                                lhsT=w1cur[f][:, m * 128 : (m + 1) * 128],
                                rhs=curT[f][:, cols],
                                start=(f == 0),
                                stop=False,
                            )
                        for f in range(2):
                            nc.tensor.matmul(
                                out=ph[:],
                                lhsT=w1cur[2 + f][:, m * 128 : (m + 1) * 128],
                                rhs=mt[f][:],
                                start=False,
                                stop=(f == 1),
                            )
                        nc.scalar.activation(
                            out=ht[m][:, cols],
                            in_=ph[:],
                            func=mybir.ActivationFunctionType.Tanh,
                            bias=b1t[e][:, m : m + 1],
                        )

                # phase 3: O = H W2 + b2 ; combine into cur (residual accum)
                for t in range(n_tiles):
                    po = ps_mm2.tile([128, D], F32, tag="mm2", space="PSUM")
                    for m in range(4):
                        nc.tensor.matmul(
                            out=po[:],
                            lhsT=ht[m][:, t * 128 : (t + 1) * 128],
                            rhs=w2cur[m][:],
                            start=(m == 0),
                            stop=False,
                        )
                    nc.tensor.matmul(
                        out=po[:], lhsT=ones_col[:], rhs=b2t[e][:], start=False, stop=True
                    )
                    ei = EXPERTS.index(e)
                    geff = gatep.tile([128, 1], F32, tag="gf", bufs=3, name="geff")
                    nc.vector.tensor_tensor(
                        out=geff[:],
                        in0=gate_tiles[t][:, ei : ei + 1],
                        in1=maskt[e][:, t : t + 1],
                        op=OP.mult,
                    )
                    nc.vector.scalar_tensor_tensor(
                        out=cur_tiles[t][:],
                        in0=po[:],
                        scalar=geff[:, 0:1],
                        in1=cur_tiles[t][:],
                        op0=OP.mult,
                        op1=OP.add,
                    )

            for t in range(n_tiles):
                nc.sync.dma_start(
                    out=out[t * 128 : (t + 1) * 128, :], in_=cur_tiles[t][:]
                )

    nc.compile()
    return nc


_MODEL_CACHE = {}


def _get_model(n_tiles=N_TILES):
    if n_tiles not in _MODEL_CACHE:
        _MODEL_CACHE[n_tiles] = build_model(n_tiles)
    return _MODEL_CACHE[n_tiles]


def kernel(**inputs):
    cell_indices = np.asarray(inputs["cell_indices"])
    full = np.asarray(inputs["full_lattice_states"], dtype=np.float32)
    idx = {e: np.asarray(inputs[f"{e}_idx"], dtype=np.int64) for e in EXPERTS}
    counts = {e: np.asarray(inputs[f"{e}_counts"], dtype=np.int64) for e in EXPERTS}
    weights = _prep_weights({k: np.asarray(v) for k, v in inputs.items()})

    nc = _get_model()
    in_maps = [
        _build_core_inputs(c, cell_indices.astype(np.int64), full, idx, counts, weights)
        for c in range(N_CORES)
    ]
    res = bass_utils.run_bass_kernel_spmd(
        nc, in_maps, core_ids=list(range(N_CORES)),
        trace=bool(os.environ.get("KERNEL_TRACE")),
    )
    out = np.concatenate([res.results[c]["out"] for c in range(N_CORES)], axis=0)
    if os.environ.get("KERNEL_TRACE"):
        kernel.last_result = res
    return out
